# revision 16
# baseline (speedup 1.0000x reference)
"""AttnBlock++1d Trainium2 kernel.

B=8, C=512, T=1024, H=8 heads (Ch=64), 32 groupnorm groups.
Sharding: data-parallel over batch, one batch element per NeuronCore (8 cores).

Per-core design (all matmuls bf16 operands / fp32 PSUM accumulation):
  - GroupNorm: bn_stats per channel, group-aggregate via a [C,32] selection
    matmul (M=1 rows -> group means on free dim), rsqrt = ACT sqrt + exact
    reciprocal + one Newton step, per-channel scale/bias scattered back to
    partitions via K=1 transpose matmuls, applied with tensor_scalar.
  - q = (0.125*W0)^T h + 0.125*b0, k = W1^T h + b1 in [C, T] layout;
    v^T = h^T W2 in [T, C] layout (so attention needs no transposes).
  - Scores computed transposed, S^T[i,t] = k^T q per head; softmax without
    max-subtraction (scores are O(30), exp stays in fp32 range); exp on ACT
    reading 4 PSUM banks at once; E stored bf16.
  - AV: a[c,t] = sum_i vT_aug[i, c|1]^T E[i,t] with a ones column appended to
    v^T so row 64 of the PSUM output is the softmax denominator.
  - Normalize: reciprocal_approx_fast + DRAM-bounce partition broadcast of
    the sums row, tensor_tensor multiply, + b2 (softmax weights sum to 1 so
    the v bias is a plain post-add).
  - out = (x + b3) + W3^T a, with x+b3 folded on host.
"""

import numpy as np
import ml_dtypes

B, C, T = 8, 512, 1024
H = 8
CH = C // H  # 64
G = 32  # groupnorm groups
GS = C // G  # 16 channels per group
EPS = 1e-6
NT = C // 128  # 4 channel tiles
IT = T // 128  # 8 i-tiles
NCORES = 8

_bf16 = ml_dtypes.bfloat16


def _build_nc():
    import concourse.bass as bass
    import concourse.tile as tile
    from concourse import bacc, mybir

    f32 = mybir.dt.float32
    bf16 = mybir.dt.bfloat16
    AF = mybir.ActivationFunctionType
    OP = mybir.AluOpType

    nc = bacc.Bacc("TRN2", target_bir_lowering=False, debug=False)

    x_d = nc.dram_tensor("x", [C, T], f32, kind="ExternalInput").ap()
    xb3_d = nc.dram_tensor("xb3", [C, T], f32, kind="ExternalInput").ap()
    w_d = [
        nc.dram_tensor(f"w{i}", [C, C], bf16, kind="ExternalInput").ap()
        for i in range(4)
    ]
    bqk_d = nc.dram_tensor("bqk", [128, 2 * NT], f32, kind="ExternalInput").ap()
    b2h_d = nc.dram_tensor("b2h", [CH, H], f32, kind="ExternalInput").ap()
    gb_d = nc.dram_tensor("gb", [128, 2 * NT], f32, kind="ExternalInput").ap()
    p_d = nc.dram_tensor("pmat", [128, 128], f32, kind="ExternalInput").ap()
    out_d = nc.dram_tensor("out", [C, T], f32, kind="ExternalOutput").ap()

    with tile.TileContext(nc) as tc:
        _emit(nc, tc, bass, mybir, f32, bf16, AF, OP,
              x_d, xb3_d, w_d, bqk_d, b2h_d, gb_d, p_d, out_d)
    nc.compile()
    return nc


def _emit(nc, tc, bass, mybir, f32, bf16, AF, OP,
          x_d, xb3_d, w_d, bqk_d, b2h_d, gb_d, p_d, out_d):
    from contextlib import ExitStack

    ctx = ExitStack()
    with ctx:
        persist = ctx.enter_context(tc.tile_pool(name="persist", bufs=1))
        small = ctx.enter_context(tc.tile_pool(name="small", bufs=2))
        epool = ctx.enter_context(tc.tile_pool(name="epool", bufs=8))
        ostp = ctx.enter_context(tc.tile_pool(name="ostp", bufs=2))
        dram = ctx.enter_context(tc.tile_pool(name="dram", bufs=2, space="DRAM"))

        # ---- persistent SBUF tiles ----
        x_sb = persist.tile([128, NT * T], f32, tag="x")
        xb3_sb = persist.tile([128, NT * T], f32, tag="xb3")
        h_sb = persist.tile([128, NT * T], bf16, tag="h")
        q_sb = persist.tile([128, NT * T], bf16, tag="q")
        k_sb = persist.tile([128, NT * T], bf16, tag="k")
        vt_sb = persist.tile([128, IT * H * (CH + 1)], bf16, tag="vt")  # 8*520
        a_sb = persist.tile([128, NT * T], bf16, tag="a")
        w_sb = [[persist.tile([128, C], bf16, tag=f"w{i}_{j}", name=f"w{i}_{j}")
                 for j in range(NT)] for i in range(4)]
        p_sb = persist.tile([128, 128], f32, tag="pmat")
        bqk_sb = persist.tile([128, 2 * NT], f32, tag="bqk")
        b2h_sb = persist.tile([CH, H], f32, tag="b2h")
        gb_sb = persist.tile([128, 2 * NT], f32, tag="gb")
        scb_sb = persist.tile([128, 2 * NT], f32, tag="scb")

        # ---- input DMAs ----
        for j in range(NT):
            nc.sync.dma_start(x_sb[:, j * T:(j + 1) * T], x_d[j * 128:(j + 1) * 128, :])
        nc.sync.dma_start(p_sb[:], p_d[:])
        nc.sync.dma_start(bqk_sb[:], bqk_d[:])
        nc.sync.dma_start(b2h_sb[:], b2h_d[:])
        nc.sync.dma_start(gb_sb[:], gb_d[:])
        for i in range(4):
            for j in range(NT):
                nc.sync.dma_start(w_sb[i][j][:], w_d[i][j * 128:(j + 1) * 128, :])
        for j in range(NT):
            nc.sync.dma_start(xb3_sb[:, j * T:(j + 1) * T],
                              xb3_d[j * 128:(j + 1) * 128, :])

        # =================== GroupNorm ===================
        gn_ctx = ExitStack()
        gn_ps = gn_ctx.enter_context(tc.tile_pool(name="gn_ps", bufs=1, space="PSUM"))

        me_cols = persist.tile([128, 2 * NT], f32, tag="me")  # (mean, var+mean^2) per tile
        for j in range(NT):
            stats = small.tile([128, 2, 6], f32, tag="bnst")
            xv = x_sb[:, j * T:(j + 1) * T].rearrange("p (a b) -> p a b", a=2)
            nc.vector.bn_stats(stats[:, 0, :], xv[:, 0, :])
            nc.vector.bn_stats(stats[:, 1, :], xv[:, 1, :])
            mv = small.tile([128, 2], f32, tag="bnmv")
            nc.vector.bn_aggr(mv[:], stats[:])
            # me_cols[:, 2j] = mean, me_cols[:, 2j+1] = var + mean^2
            nc.vector.tensor_copy(me_cols[:, 2 * j:2 * j + 1], mv[:, 0:1])
            nc.vector.tensor_tensor(
                out=me_cols[:, 2 * j + 1:2 * j + 2], in0=mv[:, 0:1], in1=mv[:, 0:1],
                op=OP.mult)
            nc.vector.tensor_tensor(
                out=me_cols[:, 2 * j + 1:2 * j + 2],
                in0=me_cols[:, 2 * j + 1:2 * j + 2], in1=mv[:, 1:2], op=OP.add)

        # Group-aggregate AND broadcast back to channels in one matmul per
        # tile: P[c, c'] = 1/16 if same group. Output = per-channel group
        # (mean, var+mean^2), already replicated across each group.
        ge_sb = persist.tile([128, 2 * NT], f32, tag="ge")
        for j in range(NT):
            ge_ps = gn_ps.tile([128, 2], f32, tag="ge_ps")
            nc.tensor.matmul(ge_ps[:], p_sb[:], me_cols[:, 2 * j:2 * j + 2],
                             start=True, stop=True)
            nc.vector.tensor_copy(ge_sb[:, 2 * j:2 * j + 2], ge_ps[:])

        # batched over the 4 tiles with stride-2 column views [128, NT]
        ge_v = ge_sb[:].rearrange("p (j s) -> p s j", s=2)
        mu_all, e_all = ge_v[:, 0, :], ge_v[:, 1, :]
        veps = small.tile([128, NT], f32, tag="veps")
        nc.vector.tensor_tensor(out=veps[:], in0=mu_all, in1=mu_all, op=OP.mult)
        nc.vector.tensor_tensor(out=veps[:], in0=e_all, in1=veps[:], op=OP.subtract)
        nc.vector.tensor_scalar_add(out=veps[:], in0=veps[:], scalar1=float(EPS))
        # rsig = rsqrt(veps) with one Newton refinement
        sig = small.tile([128, NT], f32, tag="sig")
        nc.scalar.activation(sig[:], veps[:], AF.Sqrt)
        rsig0 = small.tile([128, NT], f32, tag="rsig0")
        nc.vector.reciprocal(rsig0[:], sig[:])
        tnw = small.tile([128, NT], f32, tag="tnw")
        nc.vector.tensor_tensor(out=tnw[:], in0=rsig0[:], in1=rsig0[:], op=OP.mult)
        nc.vector.tensor_tensor(out=tnw[:], in0=tnw[:], in1=veps[:], op=OP.mult)
        nc.vector.tensor_scalar(out=tnw[:], in0=tnw[:], scalar1=-0.5, scalar2=1.5,
                                op0=OP.mult, op1=OP.add)
        rsig = small.tile([128, NT], f32, tag="rsig")
        nc.vector.tensor_tensor(out=rsig[:], in0=rsig0[:], in1=tnw[:], op=OP.mult)

        # scale = gamma * rsig ; bias = beta - mu * scale  (strided scb writes)
        scb_v = scb_sb[:].rearrange("p (j s) -> p s j", s=2)
        scale_cols, bias_cols = scb_v[:, 0, :], scb_v[:, 1, :]
        nc.vector.tensor_tensor(out=scale_cols, in0=gb_sb[:, 0:NT], in1=rsig[:],
                                op=OP.mult)
        tmu = small.tile([128, NT], f32, tag="tmu")
        nc.vector.tensor_tensor(out=tmu[:], in0=mu_all, in1=scale_cols, op=OP.mult)
        nc.vector.tensor_tensor(out=bias_cols, in0=gb_sb[:, NT:2 * NT], in1=tmu[:],
                                op=OP.subtract)

        # apply: h = x * scale + bias (bf16 out)
        for j in range(NT):
            nc.vector.tensor_scalar(
                out=h_sb[:, j * T:(j + 1) * T], in0=x_sb[:, j * T:(j + 1) * T],
                scalar1=scb_sb[:, 2 * j:2 * j + 1], scalar2=scb_sb[:, 2 * j + 1:2 * j + 2],
                op0=OP.mult, op1=OP.add)

        gn_ctx.close()

        # =================== QKV projections ===================
        qkv_ctx = ExitStack()
        qkv_ps = qkv_ctx.enter_context(tc.tile_pool(name="qkv_ps", bufs=2, space="PSUM"))
        vp_ps = qkv_ctx.enter_context(tc.tile_pool(name="vp_ps", bufs=2, space="PSUM"))

        for (wi, dst, bcol0) in ((0, q_sb, 0), (1, k_sb, NT)):
            for m in range(NT):
                pp = qkv_ps.tile([128, T], f32, tag="qk")
                for ch in range(2):
                    for kk in range(NT):
                        nc.tensor.matmul(
                            pp[:, ch * 512:(ch + 1) * 512],
                            w_sb[wi][kk][:, m * 128:(m + 1) * 128],
                            h_sb[:, kk * T + ch * 512: kk * T + (ch + 1) * 512],
                            start=(kk == 0), stop=(kk == NT - 1))
                nc.vector.tensor_scalar_add(
                    out=dst[:, m * T:(m + 1) * T], in0=pp[:],
                    scalar1=bqk_sb[:, bcol0 + m: bcol0 + m + 1])

        # vT (with ones columns for the softmax denominator)
        nc.vector.memset(vt_sb[:], 1.0)
        AUG = CH + 1  # 65
        for it in range(IT):
            vp = vp_ps.tile([128, C], f32, tag="vp")
            for kk in range(NT):
                nc.tensor.matmul(
                    vp[:], h_sb[:, kk * T + it * 128: kk * T + (it + 1) * 128],
                    w_sb[2][kk][:], start=(kk == 0), stop=(kk == NT - 1))
            vt_view = vt_sb[:, it * H * AUG:(it + 1) * H * AUG].rearrange(
                "p (h c) -> p h c", h=H)
            nc.vector.tensor_copy(vt_view[:, :, 0:CH],
                                  vp[:].rearrange("p (h c) -> p h c", h=H))

        qkv_ctx.close()

        # =================== Attention (per head pair) ===================
        at_ctx = ExitStack()
        at_ps = at_ctx.enter_context(tc.tile_pool(name="at_ps", bufs=1, space="PSUM"))
        av_ps = at_ctx.enter_context(tc.tile_pool(name="av_ps", bufs=1, space="PSUM"))

        for p in range(H // 2):
            h0, h1 = 2 * p, 2 * p + 1
            av = av_ps.tile([AUG, 2 * T], f32, tag="av")  # h0: cols 0:T, h1: T:2T
            e_tiles = []
            for it in range(IT):
                st = at_ps.tile([128, 2 * T], f32, tag="st")
                for ch in range(2):
                    nc.tensor.matmul(
                        st[:, ch * 512:(ch + 1) * 512],
                        k_sb[0:64, p * T + it * 128: p * T + (it + 1) * 128],
                        q_sb[0:64, p * T + ch * 512: p * T + (ch + 1) * 512],
                        start=True, stop=True)
                    nc.tensor.matmul(
                        st[:, T + ch * 512: T + (ch + 1) * 512],
                        k_sb[64:128, p * T + it * 128: p * T + (it + 1) * 128],
                        q_sb[64:128, p * T + ch * 512: p * T + (ch + 1) * 512],
                        start=True, stop=True)
                et = epool.tile([128, 2 * T], bf16, tag="E")
                nc.scalar.activation(et[:], st[:], AF.Exp)
                e_tiles.append(et)
                base = it * H * AUG
                for ch in range(2):
                    nc.tensor.matmul(
                        av[:, ch * 512:(ch + 1) * 512],
                        vt_sb[:, base + h0 * AUG: base + h0 * AUG + AUG],
                        et[:, ch * 512:(ch + 1) * 512],
                        start=(it == 0), stop=(it == IT - 1))
                    nc.tensor.matmul(
                        av[:, T + ch * 512: T + (ch + 1) * 512],
                        vt_sb[:, base + h1 * AUG: base + h1 * AUG + AUG],
                        et[:, T + ch * 512: T + (ch + 1) * 512],
                        start=(it == 0), stop=(it == IT - 1))

            # normalize by the sums row (row 64), then place into a_sb
            sums_sb = small.tile([1, 2 * T], f32, tag="sums")
            nc.vector.tensor_copy(sums_sb[:], av[CH:CH + 1, :])
            r_sb = small.tile([1, 2 * T], f32, tag="r_sb")
            nc.vector.reciprocal_approx_fast(out=r_sb[:], in_=sums_sb[:])
            scr = dram.tile([1, 2 * T], f32, tag="rscr")
            nc.sync.dma_start(scr[:], r_sb[:])
            r_bc = small.tile([64, 2 * T], f32, tag="r_bc")
            nc.gpsimd.dma_start(
                r_bc[:], bass.AP(tensor=scr.tensor, offset=scr[:].offset,
                                 ap=[[0, 64], [1, 2 * T]]))
            for hh, colofs in ((h0, 0), (h1, T)):
                stage = small.tile([64, T], bf16, tag="stage")
                nc.vector.tensor_tensor(out=stage[:], in0=av[0:CH, colofs:colofs + T],
                                        in1=r_bc[:, colofs:colofs + T], op=OP.mult)
                stage2 = small.tile([64, T], bf16, tag="stage2")
                nc.vector.tensor_scalar_add(out=stage2[:], in0=stage[:],
                                            scalar1=b2h_sb[:, hh:hh + 1])
                rowofs = 64 * (hh % 2)
                nc.sync.dma_start(
                    a_sb[rowofs:rowofs + CH, p * T:(p + 1) * T], stage2[:])

        at_ctx.close()

        # =================== NIN3 + residual ===================
        nin_ps = ctx.enter_context(tc.tile_pool(name="nin_ps", bufs=2, space="PSUM"))
        for m in range(NT):
            pp = nin_ps.tile([128, T], f32, tag="nin")
            for ch in range(2):
                for kk in range(NT):
                    nc.tensor.matmul(
                        pp[:, ch * 512:(ch + 1) * 512],
                        w_sb[3][kk][:, m * 128:(m + 1) * 128],
                        a_sb[:, kk * T + ch * 512: kk * T + (ch + 1) * 512],
                        start=(kk == 0), stop=(kk == NT - 1))
            ost = ostp.tile([128, T], f32, tag="ost")
            nc.vector.tensor_tensor(out=ost[:], in0=pp[:],
                                    in1=xb3_sb[:, m * T:(m + 1) * T], op=OP.add)
            nc.sync.dma_start(out_d[m * 128:(m + 1) * 128, :], ost[:])


def _host_inputs(inputs):
    """Build the per-core in_maps from the full problem inputs."""
    x = np.ascontiguousarray(inputs["x"], dtype=np.float32)
    gamma = np.asarray(inputs["gamma"], dtype=np.float32)
    beta = np.asarray(inputs["beta"], dtype=np.float32)
    scale = np.float32(CH ** -0.5)  # 0.125, exact power of two

    w0 = (np.asarray(inputs["W0"], dtype=np.float32) * scale).astype(_bf16)
    w1 = np.asarray(inputs["W1"], dtype=np.float32).astype(_bf16)
    w2 = np.asarray(inputs["W2"], dtype=np.float32).astype(_bf16)
    w3 = np.asarray(inputs["W3"], dtype=np.float32).astype(_bf16)

    b0 = np.asarray(inputs["b0"], dtype=np.float32) * scale
    b1 = np.asarray(inputs["b1"], dtype=np.float32)
    b2 = np.asarray(inputs["b2"], dtype=np.float32)
    b3 = np.asarray(inputs["b3"], dtype=np.float32)

    bqk = np.concatenate([b0.reshape(NT, 128).T, b1.reshape(NT, 128).T], axis=1)
    bqk = np.ascontiguousarray(bqk, dtype=np.float32)  # [128, 8]
    b2h = np.ascontiguousarray(b2.reshape(H, CH).T, dtype=np.float32)  # [64, 8]

    # block-diagonal group-averaging matrix: P[c, c'] = 1/16 if same group
    cc = np.arange(128)
    pmat = (cc[:, None] // GS == cc[None, :] // GS).astype(np.float32) / GS

    gb = np.concatenate([gamma.reshape(NT, 128).T, beta.reshape(NT, 128).T], axis=1)
    gb = np.ascontiguousarray(gb, dtype=np.float32)  # [128, 8]

    common = {
        "w0": w0, "w1": w1, "w2": w2, "w3": w3,
        "bqk": bqk, "b2h": b2h, "gb": gb, "pmat": pmat,
    }
    in_maps = []
    for b in range(NCORES):
        m = dict(common)
        m["x"] = np.ascontiguousarray(x[b])
        m["xb3"] = np.ascontiguousarray(x[b] + b3[:, None])
        in_maps.append(m)
    return in_maps


def kernel(**inputs) -> np.ndarray:
    from concourse.bass_utils import run_bass_kernel_spmd

    nc = _build_nc()
    in_maps = _host_inputs(inputs)
    res = run_bass_kernel_spmd(nc, in_maps, core_ids=list(range(NCORES)))
    out = np.stack([np.asarray(r["out"], dtype=np.float32) for r in res.results])
    return out
